# revision 1
# baseline (speedup 1.0000x reference)
"""ALayer kernel for 8 TRN2 NeuronCores — pure data parallel over batch.

Per-core shard: 4 images of [256, 56, 56].
  h  = relu(conv3x3(x_in, w1))      # 256 -> 16 ch
  A  = sigmoid(conv3x3(h, w2))      # 16 -> 1 ch
  out = x_out * box3x3(A)           # broadcast over 256 ch

v6 design — column-tiled TensorEngine (4 concurrent 32-col subarray strips):
  conv1 uses an interleaved row-phase layout: data row y (padded coords
  1..56) maps to phase j=(y-1)%4, supergroup s=(y-1)//28, slot
  r=((y-1)%28)//4, i.e. y = 1 + 28s + 4r + j.
  conv1: per supergroup s, 18 accumulation rounds (2 K-chunks x 9 taps);
         each round = 4 concurrent col-tiled matmuls (tile_position=
         (0,32j)), M=16 out-ch, N=7x56 pixels of phase j.
  relu:  one [128,7,56] PSUM->SBUF activation per supergroup into ht
         [128, 14, 58] (partition 32j+m = (phase j, ch m); cols 0,57 zero).
  h-gather: 4 engine copies (partition-base shift) ht[32j:32j+16] ->
         h1[0:16] plane rows 1+j::4 — descriptor-free, no DMA latency.
  conv2: col-tile j = BLOCK b=4s+j (7 consecutive rows), 9 rounds (dy,dx),
         K=16 windows from the h1 plane; M=1 at psum[32j].
  sigmoid: one [128,7,56] activation per supergroup -> a_tmp [128,2,7,58]
         (58-wide rows, cols 0,57 zero guards).
  a9:    9 pre-shifted copies of the A plane: 6 scatter-DMAs ((c,s):
         4 contiguous 406-elem runs each) into a9[3:6] + 2 row-shift
         fills.  Run col-shift (1-c) makes single-element overflows land
         exactly on zero guards.
  box:   ONE K=9 matmul per 7-row block (lhsT = ones[9,128]) produces
         box3x3(A) broadcast to 128 partitions in PSUM.
  mul:   DVE multiplies with x_out (bf16 out, cast to fp32 on host).
"""

import numpy as np
import ml_dtypes

import concourse.bass as bass
import concourse.tile as tile
import concourse.mybir as mybir
from concourse import bacc
from concourse.bass_utils import run_bass_kernel_spmd

BF16 = mybir.dt.bfloat16
FP8 = mybir.dt.float8e4
F32 = mybir.dt.float32

B, C, H, W = 32, 256, 56, 56
NCORES = 8
BL = B // NCORES          # images per core
KCH = 2                   # 256 = 2 chunks of 128
HP = H + 2                # padded plane side (58)
HW = H * W                # 3136
PL = HP * HP              # 3364

_cache = {}


def _build():
    nc = bacc.Bacc("TRN2", target_bir_lowering=False, debug=False)

    xin_d = nc.dram_tensor("xin", [BL, KCH, 128, PL], FP8, kind="ExternalInput").ap()
    xout_d = nc.dram_tensor("xout", [BL, 128, KCH, HW], BF16, kind="ExternalInput").ap()
    w1_d = nc.dram_tensor("w1t", [128, KCH, 9, 16], FP8, kind="ExternalInput").ap()
    w2_d = nc.dram_tensor("w2t", [128, 9], BF16, kind="ExternalInput").ap()
    out_d = nc.dram_tensor("out", [BL, 128, KCH, HW], BF16, kind="ExternalOutput").ap()

    with tile.TileContext(nc) as tc:
        with (
            tc.tile_pool(name="const", bufs=1) as constp,
            tc.tile_pool(name="xpad", bufs=3) as xpadp,
            tc.tile_pool(name="h1", bufs=3) as h1p,
            tc.tile_pool(name="at", bufs=3) as atp,
            tc.tile_pool(name="a9", bufs=3) as a9p,
            tc.tile_pool(name="xo", bufs=4) as xop,
            tc.tile_pool(name="ot", bufs=2) as otp,
            tc.tile_pool(name="ps_h", bufs=2, space="PSUM") as ps_h,
            tc.tile_pool(name="ps_a", bufs=2, space="PSUM") as ps_a,
            tc.tile_pool(name="ps_b", bufs=3, space="PSUM") as ps_b,
            tc.tile_pool(name="ps_w", bufs=1, space="PSUM") as ps_w,
        ):
            w1sb = constp.tile([128, KCH, 9, 16], FP8)
            w2sb = constp.tile([128, 9], BF16)
            nc.scalar.dma_start(w1sb[:], w1_d[:])
            nc.scalar.dma_start(w2sb[:], w2_d[:])
            ones9 = constp.tile([9, 128], BF16)
            nc.vector.memset(ones9[:], 1.0)
            w1flat = w1sb.rearrange("p k t m -> p (k t m)")

            def warm(n):
                # keep-warm matmuls on const data; prevents HAM re-throttle
                for _ in range(n):
                    wp = ps_w.tile([16, 288], F32)
                    nc.tensor.matmul(
                        wp[:],
                        w1sb[:, 0, 0, :],
                        w1flat,
                        start=True,
                        stop=True,
                        skip_group_check=True,
                    )

            h1s, a9s, xos = {}, {}, {}

            def stage_front(img):
                # ---- load x_in (pre-padded fp8); halves, k0 first ----
                xpad = xpadp.tile([128, KCH, HP, HP], FP8)
                xpf = xpad.rearrange("p k r w -> p k (r w)")
                MID = 30 * HP
                for k in range(KCH):
                    nc.sync.dma_start(xpf[:, k, 0:MID], xin_d[img, k, :, 0:MID])
                for k in range(KCH):
                    nc.sync.dma_start(xpf[:, k, MID:PL], xin_d[img, k, :, MID:PL])

                # ---- conv1 -> psum; relu writes the h1 plane directly ----
                h1 = h1p.tile([16, HP, HP], BF16)
                h1s[img] = h1
                if img < 3:
                    nc.vector.memset(h1[:, 0, :], 0.0)
                    nc.vector.memset(h1[:, 57, :], 0.0)
                    nc.vector.memset(h1[:, :, 0], 0.0)
                    nc.vector.memset(h1[:, :, 57], 0.0)
                for s in range(2):
                    ps = ps_h.tile([128, 7, 56], F32)
                    rnd = 0
                    for k in range(KCH):
                        for t in range(9):
                            dy, dx = t // 3, t % 3
                            for j in range(4):
                                rs = 28 * s + j + dy
                                nc.tensor.matmul(
                                    ps[32 * j : 32 * j + 16],
                                    w1sb[:, k, t, :],
                                    xpad[:, k, rs : rs + 25 : 4, dx : dx + 56],
                                    start=(rnd == 0),
                                    stop=(rnd == 17),
                                    tile_position=(0, 32 * j),
                                    skip_group_check=True,
                                )
                            rnd += 1
                    for j in range(4):
                        r0 = 1 + 28 * s + j
                        dst = h1[:, r0 : r0 + 25 : 4, 1:57]
                        if j < 2:
                            nc.scalar.activation(
                                dst,
                                ps[32 * j : 32 * j + 16],
                                mybir.ActivationFunctionType.Relu,
                            )
                        else:
                            nc.vector.tensor_scalar_max(
                                dst, ps[32 * j : 32 * j + 16], 0.0
                            )

                # prefetch x_out
                xo = xop.tile([128, KCH, HW], BF16)
                xos[img] = xo
                nc.sync.dma_start(xo[:], xout_d[img])

            def stage_mid(img):
                h1 = h1s[img]
                at = atp.tile([128, 2, 7, HP], BF16)
                a9 = a9p.tile([9, HP, HP], BF16)
                a9s[img] = a9
                a9f = a9.rearrange("p r w -> p (r w)")
                if img < 3:
                    nc.vector.memset(at[:, :, :, 0], 0.0)
                    nc.vector.memset(at[:, :, :, 57], 0.0)
                    nc.vector.memset(a9[:, 0, :], 0.0)
                    nc.vector.memset(a9[:, 57, :], 0.0)
                    nc.vector.memset(a9[:, :, 0:2], 0.0)
                    nc.vector.memset(a9[:, :, 56:58], 0.0)
                for s in range(2):
                    ps = ps_a.tile([128, 7, 56], F32)
                    rnd = 0
                    for dy in range(3):
                        for dx in range(3):
                            for j in range(4):
                                b = 4 * s + j
                                nc.tensor.matmul(
                                    ps[32 * j : 32 * j + 1],
                                    w2sb[0:16, rnd : rnd + 1],
                                    h1[:, 7 * b + dy : 7 * b + dy + 7, dx : dx + 56],
                                    start=(rnd == 0),
                                    stop=(rnd == 8),
                                    tile_position=(0, 32 * j),
                                    skip_group_check=True,
                                )
                            rnd += 1
                    nc.scalar.activation(
                        at[:, s, :, 1:57],
                        ps[:],
                        mybir.ActivationFunctionType.Sigmoid,
                    )
                    if s == 1:
                        for c in range(3):
                            for s2 in range(2):
                                st = (1 + 28 * s2) * HP + (1 - c)
                                nc.gpsimd.dma_start(
                                    a9f[3 + c : 4 + c, st : st + 1624],
                                    at[0:128:32, s2],
                                )
                        nc.gpsimd.dma_start(
                            a9f[0:3, HP : 57 * HP], a9f[3:6, 0 : 56 * HP]
                        )
                        nc.gpsimd.dma_start(
                            a9f[6:9, HP : 57 * HP], a9f[3:6, 2 * HP : PL]
                        )

            def stage_back(img):
                a9 = a9s[img]
                xo = xos[img]
                ot = otp.tile([128, KCH, HW], BF16)
                for R in range(8):
                    ps = ps_b.tile([128, 7, 56], F32)
                    nc.tensor.matmul(
                        ps[:],
                        ones9[:],
                        a9[:, 1 + 7 * R : 8 + 7 * R, 1:57],
                        start=True,
                        stop=True,
                    )
                    psb = (
                        ps.rearrange("p r w -> p (r w)")
                        .unsqueeze(1)
                        .broadcast_to([128, KCH, 392])
                    )
                    nc.vector.tensor_mul(
                        ot[:, :, 392 * R : 392 * (R + 1)],
                        xo[:, :, 392 * R : 392 * (R + 1)],
                        psb,
                    )
                for k in range(KCH):
                    for hh in range(2):
                        nc.scalar.dma_start(
                            out_d[img, :, k, 1568 * hh : 1568 * (hh + 1)],
                            ot[:, k, 1568 * hh : 1568 * (hh + 1)],
                        )

            # software pipeline: F(i) || M(i-1) || B(i-2)
            warm(24)
            stage_front(0)
            stage_front(1)
            stage_mid(0)
            stage_front(2)
            stage_mid(1)
            warm(10)
            stage_back(0)
            stage_front(3)
            stage_mid(2)
            warm(6)
            stage_back(1)
            stage_mid(3)
            warm(10)
            stage_back(2)
            warm(10)
            stage_back(3)

    nc.compile()
    return nc


def _prep_shards(x_in, x_out, w1, w2):
    bf16 = ml_dtypes.bfloat16
    fp8 = ml_dtypes.float8_e4m3
    # w1t[c, k, t, m] = w1[m, 128k + c, dy, dx],  t = 3*dy + dx
    w1t = np.ascontiguousarray(
        w1.reshape(16, KCH, 128, 9).transpose(2, 1, 3, 0)
    ).astype(fp8)
    # w2t[32g + c, t] = w2[0, c, dy, dx] replicated at 4 partition bases
    w2t = np.zeros((128, 9), dtype=bf16)
    for g in range(4):
        w2t[32 * g : 32 * g + 16, :] = w2[0].reshape(16, 9).astype(bf16)
    xi = np.zeros((NCORES, BL, KCH, 128, HP, HP), dtype=fp8)
    xi[..., 1 : 1 + H, 1 : 1 + W] = (
        x_in.reshape(NCORES, BL, KCH, 128, H, W).astype(fp8)
    )
    xi = xi.reshape(NCORES, BL, KCH, 128, PL)
    # xout[img, c_partition, k, hw]
    xo = np.ascontiguousarray(
        x_out.reshape(NCORES, BL, KCH, 128, HW).transpose(0, 1, 3, 2, 4)
    ).astype(bf16)
    return [
        {
            "xin": np.ascontiguousarray(xi[i]),
            "xout": xo[i],
            "w1t": w1t,
            "w2t": w2t,
        }
        for i in range(NCORES)
    ]


def _run(in_maps, trace=False):
    if "nc" not in _cache:
        _cache["nc"] = _build()
    return run_bass_kernel_spmd(
        _cache["nc"], in_maps, core_ids=list(range(NCORES)), trace=trace
    )


def kernel(x_in, x_out, w1, w2, _trace=False):
    in_maps = _prep_shards(
        np.asarray(x_in, dtype=np.float32),
        np.asarray(x_out, dtype=np.float32),
        np.asarray(w1, dtype=np.float32),
        np.asarray(w2, dtype=np.float32),
    )
    res = _run(in_maps, trace=_trace)
    # out[img, c_partition, k, hw] bf16 -> [B, C, H, W] fp32
    out = np.stack([res.results[i]["out"] for i in range(NCORES)])
    kernel.last_exec_time_ns = res.exec_time_ns
    out = out.astype(np.float32).transpose(0, 1, 3, 2, 4)
    return out.reshape(B, C, H, W)

